# revision 2
# baseline (speedup 1.0000x reference)
"""Trainium2 Bass kernel for a 2-layer LSTM (B=64, T=256, I=512, H=1024).

Strategy: gate-column tensor parallelism across 8 cores. Core c owns a
128-wide H-chunk of all four gates (i,f,o,g) for both layers. The scan
runs both layers interleaved (layer 1 lags layer 0 by one tick) so the
layer-0 output feeds layer 1 straight out of the per-tick AllGather of
transposed hidden-state chunks. Matmuls run in bf16 (PSUM accumulates
f32); all cell math is f32.
"""

import numpy as np

B, T_FULL, I, H, L = 64, 256, 512, 1024, 2
NCORES = 8
HC = H // NCORES          # 128 per-core H chunk
GC = 4 * HC               # 512 gate columns per core
KH = H // 128             # 8 contraction chunks over H
KI = I // 128             # 4 contraction chunks over I

_GATE_ORDER = (0, 1, 3, 2)  # local col order [i|f|o|g]; ref splits gates as i,f,g,o


def _slice_cols(w, c):
    """Per-core gate-column slice of a [K, 4H] weight, local order [i|f|o|g]."""
    return np.concatenate(
        [w[:, g * H + c * HC: g * H + (c + 1) * HC] for g in _GATE_ORDER], axis=1
    )


def build_nc(T):
    import concourse.bass as bass
    import concourse.bacc as bacc
    import concourse.mybir as mybir
    import concourse.tile as tile

    f32 = mybir.dt.float32
    bf16 = mybir.dt.bfloat16
    SIG = mybir.ActivationFunctionType.Sigmoid
    TANH = mybir.ActivationFunctionType.Tanh

    nc = bacc.Bacc(
        "TRN2",
        target_bir_lowering=False,
        debug=False,
        enable_asserts=False,
        num_devices=NCORES,
    )

    # ---- DRAM I/O (per-core tensors; host pre-slices weights per core) ----
    xT = nc.dram_tensor("xT", [I, T, B], bf16, kind="ExternalInput").ap()
    wih0 = nc.dram_tensor("wih0", [I, GC], bf16, kind="ExternalInput").ap()
    whh0 = nc.dram_tensor("whh0", [H, GC], bf16, kind="ExternalInput").ap()
    wih1 = nc.dram_tensor("wih1", [H, GC], bf16, kind="ExternalInput").ap()
    whh1 = nc.dram_tensor("whh1", [H, GC], bf16, kind="ExternalInput").ap()
    bias = nc.dram_tensor("bias", [1, 2 * GC], bf16, kind="ExternalInput").ap()
    h0T = nc.dram_tensor("h0T", [L, H, B], bf16, kind="ExternalInput").ap()
    hinitT = nc.dram_tensor("hinitT", [L, HC, B], bf16, kind="ExternalInput").ap()
    c0 = nc.dram_tensor("c0", [L, B, HC], f32, kind="ExternalInput").ap()
    ident = nc.dram_tensor("ident", [B, B], f32, kind="ExternalInput").ap()
    onesv = nc.dram_tensor("onesv", [1, B], bf16, kind="ExternalInput").ap()

    y_out = nc.dram_tensor("y_out", [B, T, HC], f32, kind="ExternalOutput").ap()
    hT_out = nc.dram_tensor("hT_out", [L, B, HC], f32, kind="ExternalOutput").ap()
    cT_out = nc.dram_tensor("cT_out", [L, B, HC], f32, kind="ExternalOutput").ap()

    rg = [list(range(NCORES))]

    with tile.TileContext(nc) as tc:
        with (
            tc.tile_pool(name="wpool", bufs=1) as wpool,
            tc.tile_pool(name="xpool", bufs=3) as xpool,
            tc.tile_pool(name="gpool", bufs=3) as gpool,
            tc.tile_pool(name="actpool", bufs=3) as actpool,
            tc.tile_pool(name="cpool", bufs=2) as cpool,
            tc.tile_pool(name="hpool", bufs=2) as hpool,
            tc.tile_pool(name="htb", bufs=3) as htbpool,
            tc.tile_pool(name="psg", bufs=2, space="PSUM") as psg,
            tc.tile_pool(name="pst", bufs=2, space="PSUM") as pst,
            tc.tile_pool(name="dram", bufs=3, space="DRAM") as dram,
        ):
            # ---- static weights into SBUF ----
            w_ih0 = wpool.tile([128, KI, GC], bf16, tag="w_ih0")
            nc.sync.dma_start(w_ih0[:], wih0.rearrange("(k p) n -> p k n", p=128))
            w_hh0 = wpool.tile([128, KH, GC], bf16, tag="w_hh0")
            nc.sync.dma_start(w_hh0[:], whh0.rearrange("(k p) n -> p k n", p=128))
            w_ih1 = wpool.tile([128, KH, GC], bf16, tag="w_ih1")
            nc.sync.dma_start(w_ih1[:], wih1.rearrange("(k p) n -> p k n", p=128))
            w_hh1 = wpool.tile([128, KH, GC], bf16, tag="w_hh1")
            nc.sync.dma_start(w_hh1[:], whh1.rearrange("(k p) n -> p k n", p=128))
            b_sb = wpool.tile([1, 2 * GC], bf16, tag="b_sb")
            nc.sync.dma_start(b_sb[:], bias)
            id_sb = wpool.tile([B, B], f32, tag="id_sb")
            nc.sync.dma_start(id_sb[:], ident)
            ones_sb = wpool.tile([1, B], bf16, tag="ones_sb")
            nc.sync.dma_start(ones_sb[:], onesv)

            # ---- initial state ----
            g_cur = []  # gathered hT per layer: [128, KH, B] bf16
            for l in range(L):
                g = gpool.tile([128, KH, B], bf16, tag=f"g{l}")
                nc.sync.dma_start(g[:], h0T[l].rearrange("(k p) b -> p k b", p=128))
                g_cur.append(g)
            c_st = []
            for l in range(L):
                c = cpool.tile([B, HC], f32, tag=f"c{l}")
                nc.sync.dma_start(c[:], c0[l])
                c_st.append(c)

            def cell_math(l, ps, c_prev, tag):
                """sigmoid/tanh + cell update; returns (c_new, h_new) f32 [B, HC]."""
                sig = actpool.tile([B, 3 * HC], f32, tag=f"sig{tag}")
                nc.scalar.activation(sig[:], ps[:, 0: 3 * HC], SIG)
                gg = actpool.tile([B, HC], f32, tag=f"gg{tag}")
                nc.scalar.activation(gg[:], ps[:, 3 * HC: 4 * HC], TANH)
                ig = actpool.tile([B, HC], f32, tag=f"ig{tag}")
                nc.vector.tensor_mul(ig[:], sig[:, 0:HC], gg[:])
                fc = actpool.tile([B, HC], f32, tag=f"fc{tag}")
                nc.vector.tensor_mul(fc[:], sig[:, HC: 2 * HC], c_prev[:])
                c_new = cpool.tile([B, HC], f32, tag=f"c{l}")
                nc.vector.tensor_add(c_new[:], ig[:], fc[:])
                tch = actpool.tile([B, HC], f32, tag=f"tch{tag}")
                nc.scalar.activation(tch[:], c_new[:], TANH)
                h_new = hpool.tile([B, HC], f32, tag=f"h{l}")
                nc.vector.tensor_mul(h_new[:], sig[:, 2 * HC: 3 * HC], tch[:])
                return c_new, h_new

            def transpose_cast(h_new, tag):
                """[B, HC] f32 -> [HC, B] bf16 via PE transpose."""
                p = pst.tile([HC, B], f32, tag="pst")
                nc.tensor.transpose(p[:], h_new[:], id_sb[:])
                ht = htbpool.tile([HC, B], bf16, tag=f"ht{tag}")
                nc.vector.tensor_copy(ht[:], p[:])
                return ht

            # ---- main scan: ticks 0..T ----
            for t in range(T + 1):
                ht_b = [None, None]

                if t < T:  # layer 0 step t+1
                    ps0 = psg.tile([B, GC], f32, tag="ps0")
                    nc.tensor.matmul(
                        ps0[:], ones_sb[:], b_sb[:, 0:GC], start=True, stop=False
                    )
                    xk = xpool.tile([128, KI, B], bf16, tag="xk")
                    nc.sync.dma_start(
                        xk[:], xT[:, t, :].rearrange("(k p) b -> p k b", p=128)
                    )
                    for k in range(KI):
                        nc.tensor.matmul(
                            ps0[:], xk[:, k, :], w_ih0[:, k, :],
                            start=False, stop=False,
                        )
                    for k in range(KH):
                        nc.tensor.matmul(
                            ps0[:], g_cur[0][:, k, :], w_hh0[:, k, :],
                            start=False, stop=(k == KH - 1),
                        )
                    c_new0, h_new0 = cell_math(0, ps0, c_st[0], "0")
                    c_st[0] = c_new0
                    ht_b[0] = transpose_cast(h_new0, "0")
                    if t == T - 1:
                        nc.sync.dma_start(hT_out[0], h_new0[:])
                        nc.sync.dma_start(cT_out[0], c_new0[:])

                if t > 0:  # layer 1 step t (input = h0(t), gathered last tick)
                    ps1 = psg.tile([B, GC], f32, tag="ps1")
                    nc.tensor.matmul(
                        ps1[:], ones_sb[:], b_sb[:, GC: 2 * GC],
                        start=True, stop=False,
                    )
                    for k in range(KH):
                        nc.tensor.matmul(
                            ps1[:], g_cur[0][:, k, :], w_ih1[:, k, :],
                            start=False, stop=False,
                        )
                    for k in range(KH):
                        nc.tensor.matmul(
                            ps1[:], g_cur[1][:, k, :], w_hh1[:, k, :],
                            start=False, stop=(k == KH - 1),
                        )
                    c_new1, h_new1 = cell_math(1, ps1, c_st[1], "1")
                    c_st[1] = c_new1
                    nc.sync.dma_start(y_out[:, t - 1, :], h_new1[:])
                    ht_b[1] = transpose_cast(h_new1, "1")
                    if t == T:
                        nc.sync.dma_start(hT_out[1], h_new1[:])
                        nc.sync.dma_start(cT_out[1], c_new1[:])
                else:
                    # tick 0: layer-1 state is the (per-core) initial chunk
                    ht1 = htbpool.tile([HC, B], bf16, tag="ht1")
                    nc.sync.dma_start(ht1[:], hinitT[1])
                    ht_b[1] = ht1

                if t < T:  # gather h0(t+1) and h1(t) for next tick
                    ag_in = dram.tile([L, HC, B], bf16, tag="ag_in")
                    nc.sync.dma_start(ag_in[0], ht_b[0][:])
                    nc.sync.dma_start(ag_in[1], ht_b[1][:])
                    ag_out = dram.tile([NCORES, L, HC, B], bf16, tag="ag_out")
                    nc.gpsimd.collective_compute(
                        "AllGather",
                        mybir.AluOpType.bypass,
                        replica_groups=rg,
                        ins=[ag_in.opt()],
                        outs=[ag_out.opt()],
                    )
                    g_new = []
                    for l in range(L):
                        g = gpool.tile([128, KH, B], bf16, tag=f"g{l}")
                        nc.sync.dma_start(
                            g[:], ag_out[:, l, :, :].rearrange("r p b -> p r b")
                        )
                        g_new.append(g)
                    g_cur = g_new

    nc.compile()
    return nc


def kernel(x, h0, c0, W_ih_0, W_hh_0, b_0, W_ih_1, W_hh_1, b_1):
    import ml_dtypes
    from concourse.bass_utils import run_bass_kernel_spmd

    bf16 = ml_dtypes.bfloat16
    T = x.shape[1]

    x = np.asarray(x, np.float32)
    h0 = np.asarray(h0, np.float32)
    c0 = np.asarray(c0, np.float32)

    xT = np.ascontiguousarray(
        np.transpose(x, (2, 1, 0)).astype(bf16)
    )  # [I, T, B]
    h0T = np.ascontiguousarray(np.transpose(h0, (0, 2, 1)).astype(bf16))  # [L,H,B]
    ident = np.eye(B, dtype=np.float32)
    onesv = np.ones((1, B), dtype=bf16)

    in_maps = []
    for c in range(NCORES):
        in_maps.append(
            {
                "xT": xT,
                "wih0": _slice_cols(np.asarray(W_ih_0, np.float32), c).astype(bf16),
                "whh0": _slice_cols(np.asarray(W_hh_0, np.float32), c).astype(bf16),
                "wih1": _slice_cols(np.asarray(W_ih_1, np.float32), c).astype(bf16),
                "whh1": _slice_cols(np.asarray(W_hh_1, np.float32), c).astype(bf16),
                "bias": np.concatenate(
                    [
                        _slice_cols(np.asarray(b_0, np.float32)[None, :], c),
                        _slice_cols(np.asarray(b_1, np.float32)[None, :], c),
                    ],
                    axis=1,
                ).astype(bf16),
                "h0T": h0T,
                "hinitT": np.ascontiguousarray(
                    h0T[:, c * HC: (c + 1) * HC, :]
                ),
                "c0": np.ascontiguousarray(c0[:, :, c * HC: (c + 1) * HC]),
                "ident": ident,
                "onesv": onesv,
            }
        )

    nc = build_nc(T)
    res = run_bass_kernel_spmd(nc, in_maps, core_ids=list(range(NCORES)))

    y = np.concatenate([r["y_out"] for r in res.results], axis=2)
    hT = np.concatenate([r["hT_out"] for r in res.results], axis=2)
    cT = np.concatenate([r["cT_out"] for r in res.results], axis=2)
    return y, hT, cT


# revision 4
# speedup vs baseline: 197.1103x; 197.1103x over previous
"""Trainium2 Bass kernel for a 2-layer LSTM (B=64, T=256, I=512, H=1024).

Strategy: gate-column tensor parallelism across 8 cores. Core c owns a
128-wide H-chunk of all four gates (i,f,o,g) for both layers. The scan
runs both layers interleaved (layer 1 lags layer 0 by one tick) so the
layer-0 output feeds layer 1 straight out of the per-tick AllGather of
transposed hidden-state chunks. Matmuls run in bf16 (PSUM accumulates
f32); all cell math is f32.
"""

import numpy as np

B, T_FULL, I, H, L = 64, 256, 512, 1024, 2
NCORES = 8
HC = H // NCORES          # 128 per-core H chunk
GC = 4 * HC               # 512 gate columns per core
KH = H // 128             # 8 contraction chunks over H
KI = I // 128             # 4 contraction chunks over I

_GATE_ORDER = (0, 1, 3, 2)  # local col order [i|f|o|g]; ref splits gates as i,f,g,o


def _slice_cols(w, c):
    """Per-core gate-column slice of a [K, 4H] weight, local order [i|f|o|g]."""
    return np.concatenate(
        [w[:, g * H + c * HC: g * H + (c + 1) * HC] for g in _GATE_ORDER], axis=1
    )


def build_nc(T):
    import concourse.bass as bass
    import concourse.bacc as bacc
    import concourse.mybir as mybir
    import concourse.tile as tile

    f32 = mybir.dt.float32
    bf16 = mybir.dt.bfloat16
    SIG = mybir.ActivationFunctionType.Sigmoid
    TANH = mybir.ActivationFunctionType.Tanh

    nc = bacc.Bacc(
        "TRN2",
        target_bir_lowering=False,
        debug=False,
        enable_asserts=False,
        num_devices=NCORES,
    )

    # ---- DRAM I/O (per-core tensors; host pre-slices weights per core) ----
    xT = nc.dram_tensor("xT", [I, T, B], bf16, kind="ExternalInput").ap()
    wih0 = nc.dram_tensor("wih0", [I, GC], bf16, kind="ExternalInput").ap()
    whh0 = nc.dram_tensor("whh0", [H, GC], bf16, kind="ExternalInput").ap()
    wih1 = nc.dram_tensor("wih1", [H, GC], bf16, kind="ExternalInput").ap()
    whh1 = nc.dram_tensor("whh1", [H, GC], bf16, kind="ExternalInput").ap()
    bias = nc.dram_tensor("bias", [1, 2 * GC], bf16, kind="ExternalInput").ap()
    h0T = nc.dram_tensor("h0T", [L, H, B], bf16, kind="ExternalInput").ap()
    hinitT = nc.dram_tensor("hinitT", [L, HC, B], bf16, kind="ExternalInput").ap()
    c0 = nc.dram_tensor("c0", [L, B, HC], f32, kind="ExternalInput").ap()
    ident = nc.dram_tensor("ident", [B, B], f32, kind="ExternalInput").ap()
    onesv = nc.dram_tensor("onesv", [1, B], bf16, kind="ExternalInput").ap()

    y_out = nc.dram_tensor("y_out", [B, T, HC], f32, kind="ExternalOutput").ap()
    hT_out = nc.dram_tensor("hT_out", [L, B, HC], f32, kind="ExternalOutput").ap()
    cT_out = nc.dram_tensor("cT_out", [L, B, HC], f32, kind="ExternalOutput").ap()

    rg = [list(range(NCORES))]

    with tile.TileContext(nc) as tc:
        with (
            tc.tile_pool(name="wpool", bufs=1) as wpool,
            tc.tile_pool(name="xpool", bufs=4) as xpool,
            tc.tile_pool(name="gpool", bufs=4) as gpool,
            tc.tile_pool(name="actpool", bufs=4) as actpool,
            tc.tile_pool(name="cpool", bufs=2) as cpool,
            tc.tile_pool(name="hpool", bufs=2) as hpool,
            tc.tile_pool(name="htb", bufs=4) as htbpool,
            tc.tile_pool(name="psg", bufs=2, space="PSUM") as psg,
            tc.tile_pool(name="pst", bufs=2, space="PSUM") as pst,
            tc.tile_pool(name="dram", bufs=3, space="DRAM") as dram,
        ):
            # ---- static weights into SBUF ----
            w_ih0 = wpool.tile([128, KI, GC], bf16, tag="w_ih0")
            nc.sync.dma_start(w_ih0[:], wih0.rearrange("(k p) n -> p k n", p=128))
            w_hh0 = wpool.tile([128, KH, GC], bf16, tag="w_hh0")
            nc.sync.dma_start(w_hh0[:], whh0.rearrange("(k p) n -> p k n", p=128))
            w_ih1 = wpool.tile([128, KH, GC], bf16, tag="w_ih1")
            nc.sync.dma_start(w_ih1[:], wih1.rearrange("(k p) n -> p k n", p=128))
            w_hh1 = wpool.tile([128, KH, GC], bf16, tag="w_hh1")
            nc.sync.dma_start(w_hh1[:], whh1.rearrange("(k p) n -> p k n", p=128))
            b_sb = wpool.tile([1, 2 * GC], bf16, tag="b_sb")
            nc.sync.dma_start(b_sb[:], bias)
            id_sb = wpool.tile([B, B], f32, tag="id_sb")
            nc.sync.dma_start(id_sb[:], ident)
            ones_sb = wpool.tile([1, B], bf16, tag="ones_sb")
            nc.sync.dma_start(ones_sb[:], onesv)

            # ---- initial state ----
            g_cur = []  # gathered hT per layer: [128, KH, B] bf16
            for l in range(L):
                g = gpool.tile([128, KH, B], bf16, tag=f"g{l}")
                nc.sync.dma_start(g[:], h0T[l].rearrange("(k p) b -> p k b", p=128))
                g_cur.append(g)
            c_st = []
            for l in range(L):
                c = cpool.tile([B, HC], f32, tag=f"c{l}")
                nc.sync.dma_start(c[:], c0[l])
                c_st.append(c)

            def cell_math(l, ps, c_prev, tag):
                """sigmoid/tanh + cell update; returns (c_new, h_new) f32 [B, HC]."""
                sig = actpool.tile([B, 3 * HC], f32, tag=f"sig{tag}")
                nc.scalar.activation(sig[:], ps[:, 0: 3 * HC], SIG)
                gg = actpool.tile([B, HC], f32, tag=f"gg{tag}")
                nc.scalar.activation(gg[:], ps[:, 3 * HC: 4 * HC], TANH)
                ig = actpool.tile([B, HC], f32, tag=f"ig{tag}")
                nc.vector.tensor_mul(ig[:], sig[:, 0:HC], gg[:])
                fc = actpool.tile([B, HC], f32, tag=f"fc{tag}")
                nc.vector.tensor_mul(fc[:], sig[:, HC: 2 * HC], c_prev[:])
                c_new = cpool.tile([B, HC], f32, tag=f"c{l}")
                nc.vector.tensor_add(c_new[:], ig[:], fc[:])
                tch = actpool.tile([B, HC], f32, tag=f"tch{tag}")
                nc.scalar.activation(tch[:], c_new[:], TANH)
                h_new = hpool.tile([B, HC], f32, tag=f"h{l}")
                nc.vector.tensor_mul(h_new[:], sig[:, 2 * HC: 3 * HC], tch[:])
                return c_new, h_new

            def transpose_cast(h_new, tag):
                """[B, HC] f32 -> [HC, B] bf16 via PE transpose."""
                p = pst.tile([HC, B], f32, tag="pst")
                nc.tensor.transpose(p[:], h_new[:], id_sb[:])
                ht = htbpool.tile([HC, B], bf16, tag=f"ht{tag}")
                nc.vector.tensor_copy(ht[:], p[:])
                return ht

            # ---- main scan: ticks 0..T ----
            for t in range(T + 1):
                ht_b = [None, None]

                if t < T:  # layer 0 step t+1
                    ps0 = psg.tile([B, GC], f32, tag="ps0")
                    nc.tensor.matmul(
                        ps0[:], ones_sb[:], b_sb[:, 0:GC], start=True, stop=False
                    )
                    xk = xpool.tile([128, KI, B], bf16, tag="xk")
                    nc.sync.dma_start(
                        xk[:], xT[:, t, :].rearrange("(k p) b -> p k b", p=128)
                    )
                    for k in range(KI):
                        nc.tensor.matmul(
                            ps0[:], xk[:, k, :], w_ih0[:, k, :],
                            start=False, stop=False,
                        )
                    for k in range(KH):
                        nc.tensor.matmul(
                            ps0[:], g_cur[0][:, k, :], w_hh0[:, k, :],
                            start=False, stop=(k == KH - 1),
                        )
                    c_new0, h_new0 = cell_math(0, ps0, c_st[0], "0")
                    c_st[0] = c_new0
                    ht_b[0] = transpose_cast(h_new0, "0")
                    if t == T - 1:
                        nc.sync.dma_start(hT_out[0], h_new0[:])
                        nc.sync.dma_start(cT_out[0], c_new0[:])

                if t > 0:  # layer 1 step t (input = h0(t), gathered last tick)
                    ps1 = psg.tile([B, GC], f32, tag="ps1")
                    nc.tensor.matmul(
                        ps1[:], ones_sb[:], b_sb[:, GC: 2 * GC],
                        start=True, stop=False,
                    )
                    for k in range(KH):
                        nc.tensor.matmul(
                            ps1[:], g_cur[0][:, k, :], w_ih1[:, k, :],
                            start=False, stop=False,
                        )
                    for k in range(KH):
                        nc.tensor.matmul(
                            ps1[:], g_cur[1][:, k, :], w_hh1[:, k, :],
                            start=False, stop=(k == KH - 1),
                        )
                    c_new1, h_new1 = cell_math(1, ps1, c_st[1], "1")
                    c_st[1] = c_new1
                    nc.sync.dma_start(y_out[:, t - 1, :], h_new1[:])
                    ht_b[1] = transpose_cast(h_new1, "1")
                    if t == T:
                        nc.sync.dma_start(hT_out[1], h_new1[:])
                        nc.sync.dma_start(cT_out[1], c_new1[:])
                else:
                    # tick 0: layer-1 state is the (per-core) initial chunk
                    ht1 = htbpool.tile([HC, B], bf16, tag="ht1")
                    nc.sync.dma_start(ht1[:], hinitT[1])
                    ht_b[1] = ht1

                if t < T:  # gather h0(t+1) and h1(t) for next tick
                    ag_in = dram.tile([L, HC, B], bf16, tag="ag_in")
                    nc.sync.dma_start(ag_in[0], ht_b[0][:])
                    nc.sync.dma_start(ag_in[1], ht_b[1][:])
                    ag_out = dram.tile([NCORES, L, HC, B], bf16, tag="ag_out")
                    nc.gpsimd.collective_compute(
                        "AllGather",
                        mybir.AluOpType.bypass,
                        replica_groups=rg,
                        ins=[ag_in.opt()],
                        outs=[ag_out.opt()],
                    )
                    g_new = []
                    for l in range(L):
                        g = gpool.tile([128, KH, B], bf16, tag=f"g{l}")
                        nc.sync.dma_start(
                            g[:], ag_out[:, l, :, :].rearrange("r p b -> p r b")
                        )
                        g_new.append(g)
                    g_cur = g_new

    nc.compile()
    return nc


def kernel(x, h0, c0, W_ih_0, W_hh_0, b_0, W_ih_1, W_hh_1, b_1):
    import ml_dtypes
    from concourse.bass_utils import run_bass_kernel_spmd

    bf16 = ml_dtypes.bfloat16
    T = x.shape[1]

    x = np.asarray(x, np.float32)
    h0 = np.asarray(h0, np.float32)
    c0 = np.asarray(c0, np.float32)

    xT = np.ascontiguousarray(
        np.transpose(x, (2, 1, 0)).astype(bf16)
    )  # [I, T, B]
    h0T = np.ascontiguousarray(np.transpose(h0, (0, 2, 1)).astype(bf16))  # [L,H,B]
    ident = np.eye(B, dtype=np.float32)
    onesv = np.ones((1, B), dtype=bf16)

    in_maps = []
    for c in range(NCORES):
        in_maps.append(
            {
                "xT": xT,
                "wih0": _slice_cols(np.asarray(W_ih_0, np.float32), c).astype(bf16),
                "whh0": _slice_cols(np.asarray(W_hh_0, np.float32), c).astype(bf16),
                "wih1": _slice_cols(np.asarray(W_ih_1, np.float32), c).astype(bf16),
                "whh1": _slice_cols(np.asarray(W_hh_1, np.float32), c).astype(bf16),
                "bias": np.concatenate(
                    [
                        _slice_cols(np.asarray(b_0, np.float32)[None, :], c),
                        _slice_cols(np.asarray(b_1, np.float32)[None, :], c),
                    ],
                    axis=1,
                ).astype(bf16),
                "h0T": h0T,
                "hinitT": np.ascontiguousarray(
                    h0T[:, c * HC: (c + 1) * HC, :]
                ),
                "c0": np.ascontiguousarray(c0[:, :, c * HC: (c + 1) * HC]),
                "ident": ident,
                "onesv": onesv,
            }
        )

    nc = build_nc(T)
    res = run_bass_kernel_spmd(nc, in_maps, core_ids=list(range(NCORES)))

    y = np.concatenate([r["y_out"] for r in res.results], axis=2)
    hT = np.concatenate([r["hT_out"] for r in res.results], axis=2)
    cT = np.concatenate([r["cT_out"] for r in res.results], axis=2)
    return y, hT, cT
